# revision 26
# baseline (speedup 1.0000x reference)
"""NNUE HalfKP embedding-bag kernel for 8 Trainium2 NeuronCores.

Strategy (data-parallel over batch):
  - Host folds the factorized table into the main one:
        Wf[i] = ft_w.T[i] + fft_w.T[i % 640]          [40960, 512]
    and appends (ft_b + fft_b) as row 40960  ->  Wt [40961, 512].
    (Standard NNUE weight folding; the per-row bias lands in the
    segment-sum via a dedicated padded index slot.)
  - Each core handles 64 batch rows (512 / 8). Per side (stm/nstm) each
    batch's 30 indices are padded to 32: slot 30 = bias row (40960),
    slot 31 = dummy row 0 with weight 0. So a core gathers
    64*32 = 2048 rows of 512 floats per side with ONE logical
    indirect-DMA gather (chunked for pipelining).
  - Segment-sum = PE matmuls: for tile t (128 gathered rows = 4 batches)
    psum[0:64, :] += S_t.T @ G_t, where S_t is a [128, 64] selection
    matrix that is a 4-column sliding window into a constant [128, 128]
    pattern (exploits the fixed 32-rows-per-batch structure).
  - clip(x, 0, 1) fused on DVE (tensor_scalar min+max), dot with out_w
    via DVE mul + reduce_sum, sigmoid(+out_b) on ACT, DMA out [64, 1].
"""

import contextlib
import ctypes
import os
import sys
import types

import numpy as np

import concourse.bacc as bacc
import concourse.bass as bass
import concourse.mybir as mybir
from concourse.bass_utils import run_bass_kernel_spmd
from concourse.tile import TileContext


def _ensure_ntff_hook():
    """Provide antenv.axon_hooks if the container's antenv stub lacks it.

    Same ctypes NTFF-profile hook trn_agent_boot would register; without
    it run_bass_kernel_spmd(trace=True) raises ModuleNotFoundError.
    """
    try:
        import antenv.axon_hooks  # noqa: F401

        return
    except ImportError:
        pass
    try:
        import antenv
    except ImportError:
        return

    state = {"hook": None, "set": False}

    def _build():
        so = "/opt/axon/libaxon_pjrt.so"
        if not os.path.exists(so):
            return None
        lib = ctypes.CDLL(so)
        if not hasattr(lib, "axon_start_nrt_profile"):
            return None
        lib.axon_start_nrt_profile.argtypes = [
            ctypes.POINTER(ctypes.c_int64),
            ctypes.c_size_t,
        ]
        lib.axon_start_nrt_profile.restype = ctypes.c_int64
        lib.axon_stop_nrt_profile.argtypes = [ctypes.c_char_p]
        lib.axon_stop_nrt_profile.restype = ctypes.c_int64

        @contextlib.contextmanager
        def _hook(output_dir, device_ids):
            import jax

            jax.devices()
            if device_ids:
                ids = (ctypes.c_int64 * len(device_ids))(*device_ids)
                rc = lib.axon_start_nrt_profile(ids, len(device_ids))
            else:
                rc = lib.axon_start_nrt_profile(None, 0)
            if rc != 0:
                raise RuntimeError(f"axon_start_nrt_profile rc={rc}")
            try:
                yield
            finally:
                n = lib.axon_stop_nrt_profile(str(output_dir).encode())
                if n < 0:
                    raise RuntimeError(f"axon_stop_nrt_profile rc={n}")

        return _hook

    mod = types.ModuleType("antenv.axon_hooks")

    def set_axon_ntff_profile_hook(h):
        state["hook"] = h
        state["set"] = True

    def get_axon_ntff_profile_hook():
        if not state["set"]:
            state["hook"] = _build()
            state["set"] = True
        return state["hook"]

    mod.set_axon_ntff_profile_hook = set_axon_ntff_profile_hook
    mod.get_axon_ntff_profile_hook = get_axon_ntff_profile_hook
    sys.modules["antenv.axon_hooks"] = mod
    antenv.axon_hooks = mod


_ensure_ntff_hook()

# Problem constants (hardcoded per contract; must match setup_inputs()).
B = 512
FEATS = 30
PB = 32  # padded features per batch (30 real + bias slot + zero slot)
FT_OUT = 512
N_FEAT = 40960
N_VFEAT = 640
N_CORES = 8
BPC = B // N_CORES          # batches per core = 64
NNZ_C = BPC * PB            # padded nnz per core per side = 2048
N_TILES = NNZ_C // 128      # 16 matmul tiles per side
BIAS_ROW = N_FEAT           # table row holding ft_b + fft_b

# dtype config: "f32" is bit-safe; "bf16" halves gather bytes.
TABLE_DT = "f32"
# matmul dtype: f32 (exact, 4 cyc/row), f32r (~tf32, 1 cyc/row).
MM_DT = "f32r"

_DT = {
    "f32": mybir.dt.float32,
    "bf16": mybir.dt.bfloat16,
    "f32r": mybir.dt.float32r,
}


def _np_dt(dt):
    return np.dtype(mybir.dt.np(dt))


def _build_bass(uniform: bool, gather_chunks: int = N_TILES):
    # NOTE: HW indirect DMA consumes ONE index per partition-row of the
    # output and reads that partition's free run contiguously from
    # table[idx[p]] — so each indirect_dma_start gathers exactly 128
    # rows ([128, 1] offsets, [128, 512] out), scatter_add-style.
    """Build the per-core Bass program (SPMD: all cores run the same code).

    uniform: all `values` entries equal -> one shared [128, 128]
    sliding-window selection pattern; else dense per-(side, tile)
    [128, 64] selection matrices.
    """
    # wire_dt: dtype of the table in DRAM, the gathered tiles, and the
    # selection matrix — i.e. the matmul input dtype. The BIR verifier
    # requires fp32r matmul inputs to be produced as fp32r, so the whole
    # chain is declared fp32r (bit-identical to f32 on the wire).
    if TABLE_DT == "f32" and MM_DT == "f32r":
        wire_dt = _DT["f32r"]
    else:
        wire_dt = _DT[TABLE_DT]
    f32 = mybir.dt.float32

    nc = bacc.Bacc()

    wt = nc.declare_dram_parameter("wt", [N_FEAT + 1, FT_OUT], wire_dt, isOutput=False)
    idx = nc.declare_dram_parameter("idx", [128, 2 * N_TILES], mybir.dt.int32, isOutput=False)
    smat_w = 128 if uniform else 2 * N_TILES * BPC
    smat = nc.declare_dram_parameter("smat", [128, smat_w], wire_dt, isOutput=False)
    # wrep: per side 513 columns — 512 of out_w plus one bias column
    # (side 0: out_b, side 1: 0); pairs with a constant-1 column in h so
    # the final dot product absorbs out_b with no separate bias tensor.
    wrep = nc.declare_dram_parameter("wrep", [BPC, 2 * (FT_OUT + 1)], f32, isOutput=False)
    y = nc.declare_dram_parameter("y", [BPC, 1], f32, isOutput=True)

    blk_per_chunk = N_TILES // gather_chunks

    with TileContext(nc) as tc:
        with (
            tc.tile_pool(name="sbuf", bufs=1) as sp,
            tc.tile_pool(name="psum", bufs=1, space="PSUM") as pp,
        ):
            idx_t = sp.tile([128, 2 * N_TILES], mybir.dt.int32, tag="idx")
            nc.sync.dma_start(out=idx_t[:], in_=idx[:])
            smat_t = sp.tile([128, smat_w], wire_dt, tag="smat")
            nc.sync.dma_start(out=smat_t[:], in_=smat[:])
            w_t = sp.tile([BPC, 2 * (FT_OUT + 1)], f32, tag="wrep")
            nc.sync.dma_start(out=w_t[:], in_=wrep[:])

            # Issue all gathers first so the DMA engines stream continuously.
            gtiles = {}  # (side, chunk) -> tile [128, blk_per_chunk*512]
            for side in range(2):
                for ch in range(gather_chunks):
                    g = sp.tile(
                        [128, blk_per_chunk * FT_OUT], wire_dt, tag=f"g{side}_{ch}"
                    )
                    off = side * N_TILES + ch * blk_per_chunk
                    nc.gpsimd.indirect_dma_start(
                        out=g[:],
                        out_offset=None,
                        in_=wt[:, :],
                        in_offset=bass.IndirectOffsetOnAxis(
                            ap=idx_t[:, off : off + blk_per_chunk], axis=0
                        ),
                    )
                    gtiles[(side, ch)] = g

            psum0 = pp.tile([BPC, FT_OUT], f32, tag="ps0")
            psum1 = pp.tile([BPC, FT_OUT], f32, tag="ps1")
            psums = [psum0, psum1]
            # Dummy matmul reading only smat: absorbs the smat-DMA wait on
            # PE (the HW-decoded LDW/MM pair supports a single sync wait,
            # and the first real matmul already waits on its gather chunk).
            # The psum[:, 0:1] WAW dep orders it before the real group.
            # (fp32r ISA requires an even moving-dim count, hence N=2.)
            nc.tensor.matmul(
                out=psums[0][:, 0:2],
                lhsT=smat_t[:, 0:BPC],
                rhs=smat_t[:, 0:2],
                start=True,
                stop=True,
            )

            part = []
            for side in range(2):
                psum = psums[side]
                for t in range(N_TILES):
                    g = gtiles[(side, t // blk_per_chunk)]
                    gs = g[:, (t % blk_per_chunk) * FT_OUT : (t % blk_per_chunk + 1) * FT_OUT]
                    if uniform:
                        ls = smat_t[:, 60 - 4 * t : 124 - 4 * t]
                    else:
                        o = side * N_TILES * BPC + t * BPC
                        ls = smat_t[:, o : o + BPC]
                    nc.tensor.matmul(
                        out=psum[:, :],
                        lhsT=ls,
                        rhs=gs,
                        start=(t == 0),
                        stop=(t == N_TILES - 1),
                    )
                # h: [clip(psum, 0, 1), 1.0] — the constant-1 column pairs
                # with wrep's bias column in the dot product.
                h = sp.tile([BPC, FT_OUT + 1], f32, tag=f"h{side}")
                nc.vector.memset(h[:, FT_OUT : FT_OUT + 1], 1.0)
                nc.vector.tensor_scalar(
                    out=h[:, 0:FT_OUT],
                    in0=psum[:],
                    scalar1=1.0,
                    scalar2=0.0,
                    op0=mybir.AluOpType.min,
                    op1=mybir.AluOpType.max,
                )
                prod = sp.tile([BPC, FT_OUT + 1], f32, tag=f"prod{side}")
                # touch w_t first (absorbs the wrep-DMA wait on DVE; the
                # prod[:, 0:1] WAW dep orders it before the mul) so the
                # mul itself needs only the single DVE-engine wait.
                wsl = slice(side * (FT_OUT + 1), (side + 1) * (FT_OUT + 1))
                nc.vector.tensor_copy(
                    out=prod[:, 0:1], in_=w_t[:, wsl.start : wsl.start + 1]
                )
                nc.vector.tensor_tensor(
                    out=prod[:],
                    in0=h[:],
                    in1=w_t[:, wsl],
                    op=mybir.AluOpType.mult,
                )
                p = sp.tile([BPC, 1], f32, tag=f"part{side}")
                nc.vector.reduce_sum(out=p[:], in_=prod[:], axis=mybir.AxisListType.X)
                part.append(p)

            tot = sp.tile([BPC, 1], f32, tag="tot")
            nc.vector.tensor_add(out=tot[:], in0=part[0][:], in1=part[1][:])
            yt = sp.tile([BPC, 1], f32, tag="yt")
            nc.scalar.activation(
                out=yt[:],
                in_=tot[:],
                func=mybir.ActivationFunctionType.Sigmoid,
            )
            nc.sync.dma_start(out=y[:], in_=yt[:])

    return nc


def _prep_inputs(stm_indices, nstm_indices, values, ft_w, ft_b, fft_w, fft_b, out_w, out_b):
    """Host-side sharding/layout prep. Returns (in_maps, uniform)."""
    np_table = _np_dt(_DT[TABLE_DT])
    f32 = np.float32

    ft_wt = np.asarray(ft_w, f32).T                      # [40960, 512]
    fft_wt = np.asarray(fft_w, f32).T                    # [640, 512]
    wf = ft_wt + np.tile(fft_wt, (N_FEAT // N_VFEAT, 1))
    bias_row = (np.asarray(ft_b, f32) + np.asarray(fft_b, f32))[None, :]
    wt = np.ascontiguousarray(
        np.concatenate([wf, bias_row], axis=0).astype(np_table)
    )  # [40961, 512]

    values = np.asarray(values, f32)
    uniform = bool(np.all(values == 1.0))

    ow = np.asarray(out_w, f32).reshape(2 * FT_OUT)
    wrow = np.zeros(2 * (FT_OUT + 1), f32)
    wrow[0:FT_OUT] = ow[0:FT_OUT]
    wrow[FT_OUT] = np.asarray(out_b, f32).reshape(1)[0]
    wrow[FT_OUT + 1 : 2 * FT_OUT + 1] = ow[FT_OUT:]
    wrep = np.ascontiguousarray(np.broadcast_to(wrow[None, :], (BPC, 2 * (FT_OUT + 1))))

    cols = {
        0: np.asarray(stm_indices[1], np.int32).reshape(B, FEATS),
        1: np.asarray(nstm_indices[1], np.int32).reshape(B, FEATS),
    }
    vals_b = values.reshape(B, FEATS)

    if uniform:
        # shared sliding-window pattern: smat[k, 60 + q] has the S4 block
        ent = np.zeros(PB, f32)
        ent[:FEATS] = 1.0
        ent[FEATS] = 1.0  # bias slot
        smat_shared = np.zeros((128, 128), f32)
        for q in range(4):
            smat_shared[q * PB : (q + 1) * PB, 60 + q] = ent
        smat_shared = smat_shared.astype(np_table)

    in_maps = []
    for c in range(N_CORES):
        bsl = slice(c * BPC, (c + 1) * BPC)
        idx_core = np.zeros((128, 2 * N_TILES), np.int32)
        for side in range(2):
            idx_pad = np.zeros((BPC, PB), np.int32)
            idx_pad[:, :FEATS] = cols[side][bsl]
            idx_pad[:, FEATS] = BIAS_ROW
            # logical order j' = b*32 + r; device layout [p, blk] = j' = blk*128 + p
            idx_core[:, side * N_TILES : (side + 1) * N_TILES] = (
                idx_pad.reshape(NNZ_C).reshape(N_TILES, 128).T
            )
        if uniform:
            smat_core = smat_shared
        else:
            smat_core = np.zeros((128, 2 * N_TILES * BPC), f32)
            for side in range(2):
                ent = np.zeros((BPC, PB), f32)
                ent[:, :FEATS] = vals_b[bsl]
                ent[:, FEATS] = 1.0
                for t in range(N_TILES):
                    blk = np.zeros((128, BPC), f32)
                    for q in range(4):
                        b = 4 * t + q
                        blk[q * PB : (q + 1) * PB, b] = ent[b]
                    o = side * N_TILES * BPC + t * BPC
                    smat_core[:, o : o + BPC] = blk
            smat_core = smat_core.astype(np_table)
        in_maps.append(
            {
                "wt": wt,
                "idx": idx_core,
                "smat": smat_core,
                "wrep": wrep,
            }
        )
    return in_maps, uniform


LAST_RESULTS = None


def kernel(**inputs) -> np.ndarray:
    global LAST_RESULTS
    in_maps, uniform = _prep_inputs(**inputs)
    nc = _build_bass(uniform)
    if not nc.is_finalized():
        nc.finalize()  # runs Bacc.compile(): wait legalization + reg alloc
    res = run_bass_kernel_spmd(nc, in_maps, core_ids=list(range(N_CORES)))
    LAST_RESULTS = res
    return np.concatenate([r["y"] for r in res.results], axis=0).astype(np.float32)
